# revision 28
# baseline (speedup 1.0000x reference)
"""Trainium2 Bass kernel for nn_AggregationLayer2 (5x5 spatially-varying
neighborhood aggregation, 26 slots: 25 spatial shifts + current value).

    out[b,h,w,c] = sum_k attn[b,h,w,k] * neighbor_k(ref_value)[c]
                 + attn[b,h,w,25] * current_ref_value[b,h,w,c]

Strategy (8 NeuronCores, SPMD):
  - Shard: (batch, H-half) -> 8 shards of 64 output rows each; host ships
    ref rows with a 2-row zero halo.
  - Compute: per output row h and vertical offset di, the dj-contraction is
    a banded matmul: out_row[w,c] += sum_{w'} BandT[w',w] * ref[h+di,w',c]
    where BandT[w', w'-2..w'+2] are the 5 attn weights. The TensorEngine
    runs this as 5 stationary loads x one N=64 matmul each per row,
    accumulating in PSUM; the host-prescaled current term is added during
    the two-row PSUM evictions on DVE (bf16 out, host converts to fp32).
  - Band build: host packs, per 32-partition group, 36-wide zero-padded
    span windows (value for output col w lands at band col q' = w + 2),
    interleaved across the 5 bands at stride 5 so each partition writes
    one contiguous 360B run per row; 4 partition-aligned DMAs per chunk.
    Un-written band cells are zeroed once at kernel start on GpSimd
    (keeping DVE free for evictions) and never dirtied.
  - Pipelining: a tiny 4-row first chunk gets the PE started ~1.5us in;
    band/ref/cur DMAs stream over both HWDGE queues in need-order and
    output DMAs drain per chunk.
"""

import numpy as np
import ml_dtypes

import concourse.bass as bass
import concourse.mybir as mybir
from concourse.tile import TileContext
from concourse.tile_rust import add_dep_helper
from concourse.vector_clock import ScopedClock
from concourse import bass_utils

# ---------------------------------------------------------------------------
# Toolchain compat: this walrus build codegens at most one sync-wait command
# per instruction and rejects eq-mode waits on Drain ops. Replace the Tile
# tail barrier and split multi-waits onto standalone EventSemaphore waits.
# ---------------------------------------------------------------------------

_wsplit_counter = [0]


def _split_multi_waits(nc):
    for f in nc.m.functions:
        for bb in f.blocks:
            out = []
            changed = False
            for inst in bb.instructions:
                si = inst.sync_info
                if si is not None and len(si.on_wait) > 1:
                    waits = list(si.on_wait)
                    for w in waits[:-1]:
                        _wsplit_counter[0] += 1
                        ev = mybir.InstEventSemaphore(
                            name=f"WSPLIT-{_wsplit_counter[0]}",
                            engine=inst.engine,
                            ins=[],
                            outs=[],
                            sync_info=mybir.SyncInfo(on_wait=[w], on_update=[]),
                        )
                        out.append(ev)
                    si.on_wait = [waits[-1]]
                    changed = True
                out.append(inst)
            if changed:
                bb.instructions = out


def _drain_and_barrier_compat(self, tick_clock, wait_clock):
    nc = self.nc
    carrier = nc.sync.nop()
    wait_clock.add_sem_waits(
        carrier.ins, ScopedClock({None: tick_clock.global_clock})
    )
    waits = list(carrier.ins.sync_info.on_wait)
    if len(waits) > 1:
        carrier.ins.sync_info.on_wait = [waits[0]]
        engines = list(nc.engines.values())
        for idx, w in enumerate(waits[1:]):
            n = engines[idx % len(engines)].nop()
            n.ins.sync_info = mybir.SyncInfo(on_wait=[w], on_update=[])

    barrier_sem = nc.alloc_semaphore("tile_final_barrier")
    n_eng = len(nc.engines)
    for eng in nc.engines.values():
        eng.drain(fusable=False)
        eng.sem_inc(barrier_sem, 1)
        eng.wait_ge(barrier_sem, n_eng)
    for _ in range(4):
        nc.gpsimd.nop()
    nc.gpsimd.sem_clear(barrier_sem)

    popped = nc._tile_sem_poison_stack.pop()
    assert popped is self._sem_poison
    nc.clear_and_free_semaphores(list(self.sems.allocated().values()))


_orig_tc_exit = TileContext.__exit__


def _patched_tc_exit(self, exc_type, exc_value, traceback):
    r = _orig_tc_exit(self, exc_type, exc_value, traceback)
    if not exc_type:
        _split_multi_waits(self.nc)
    return r


def _install_tilefix():
    TileContext._drain_and_barrier = _drain_and_barrier_compat
    TileContext.__exit__ = _patched_tc_exit


_install_tilefix()


def _install_ntff_hook():
    """The image's antenv lacks axon_hooks; provide it and register the
    ctypes NTFF profiling hook so BASS_TRACE=1 yields HW exec times."""
    import sys
    import types

    if "antenv.axon_hooks" not in sys.modules:
        mod = types.ModuleType("antenv.axon_hooks")
        holder = [None]
        mod.set_axon_ntff_profile_hook = lambda h: holder.__setitem__(0, h)
        mod.get_axon_ntff_profile_hook = lambda: holder[0]
        sys.modules["antenv.axon_hooks"] = mod
        try:
            import antenv

            antenv.axon_hooks = mod
        except ImportError:
            pass
    try:
        from antenv.axon_hooks import (
            get_axon_ntff_profile_hook,
            set_axon_ntff_profile_hook,
        )

        if get_axon_ntff_profile_hook() is None:
            from trn_agent_boot.trn_boot import _ntff_profile_via_ctypes

            set_axon_ntff_profile_hook(
                _ntff_profile_via_ctypes("/opt/axon/libaxon_pjrt.so")
            )
    except Exception:
        pass

    # artifact upload needs external storage; degrade to local-only
    def _no_upload(tmpdir):
        return tmpdir

    bass_utils.upload_artifacts = _no_upload


_install_ntff_hook()

# ---------------------------------------------------------------------------
# Problem geometry (hardcoded per the harness contract)
# ---------------------------------------------------------------------------

B, H, W, C = 4, 128, 128, 64
KSLOTS = 26
NCORES = 8
HS = H // 2          # 64 output rows per shard
HALO_R = HS + 4      # 68 ref rows incl 2-row halo
NBAND = 5
BC = 132             # band col space: q' = 0..131, output w = q' - 2
MROW = NBAND * BC    # 660 band elems per row per partition
PB = 32              # partitions per band-DMA group
NG = W // PB         # 4 groups
SPAN = PB + 4        # 36-wide zero-padded window per group
# a tiny first chunk lets the PE start while the bulk input streams in;
# later chunks grow so the per-chunk zeroing (on GpSimd, during PE
# compute of earlier chunks) stays ahead of the matmul wavefront.
CHUNKS = [(0, 2), (2, 6), (8, 8), (16, 16), (32, 16), (48, 16)]
NCH = len(CHUNKS)
# rows interleaved inside each span window: DMA runs become 180*R2 elems
# (>=512B, line rate) and the descriptor count drops by R2x.
R2_OF = [2, 2, 1, 1, 1, 1]

BF16 = mybir.dt.bfloat16
F32 = mybir.dt.float32
U32 = mybir.dt.uint32

bfloat16 = ml_dtypes.bfloat16


def _build_bass():
    nc = bass.Bass()
    refhl = nc.dram_tensor("refhl", [W, HALO_R, C], BF16, kind="ExternalInput")
    curhl = nc.dram_tensor("curhl", [W, HS, C], BF16, kind="ExternalInput")
    qtot = HS * W * SPAN * NBAND
    qb = nc.dram_tensor("qb", [qtot], BF16, kind="ExternalInput")
    out = nc.dram_tensor("out", [W, HS, C], BF16, kind="ExternalOutput")

    qoffs = []
    acc = 0
    for _, n in CHUNKS:
        qoffs.append(acc)
        acc += n * W * SPAN * NBAND
    assert acc == qtot

    NWARM = 45
    with TileContext(nc) as tc:
        with (
            tc.tile_pool(name="sb", bufs=1) as pool,
            tc.tile_pool(name="ps", bufs=7, space="PSUM") as psum_pool,
            tc.tile_pool(name="pw", bufs=1, space="PSUM") as warm_pool,
        ):
            refsb = pool.tile([W, HALO_R * C], BF16, tag="refsb")
            cursb = pool.tile([W, HS * C], BF16, tag="cursb")
            outst = pool.tile([W, HS * C], BF16, tag="outst")
            wtile = pool.tile([W, 192], BF16, tag="wtile")
            bandt = [
                pool.tile([W, n * MROW], BF16, name=f"band{i}", tag=f"band{i}")
                for i, (_, n) in enumerate(CHUNKS)
            ]

            # PE warmup: zero tile + throwaway matmuls with no input deps
            # keep the HAM clock un-throttled while the first inputs
            # stream in, so the real matmul stream starts at full rate.
            nc.vector.memset(wtile[:].bitcast(U32), 0)
            ps_warm = warm_pool.tile([W, 64], F32, tag="warm")
            for _ in range(NWARM):
                nc.tensor.matmul(ps_warm[:], wtile[:, 0:128], wtile[:, 64:128],
                                 start=True, stop=True)

            # one-time zeroing of un-written band cells, split so tile
            # readiness is monotone in chunk order: DVE zeroes the lead
            # tiles (then turns to evictions), GpSimd streams through the
            # rest during PE compute.
            nc.vector.memset(bandt[0][:].bitcast(U32), 0)
            nc.vector.memset(bandt[1][:].bitcast(U32), 0)
            for ci in range(2, NCH):
                nc.gpsimd.memset(bandt[ci][:].bitcast(U32), 0)

            # Chain every DMA on its queue with ordering-only edges: the
            # scheduler's cost model otherwise reorders queue slots, and
            # the 8-lane HWDGE semaphore accounting turns any inversion
            # into a PE stall on an unrelated later transfer.
            qlast = {}

            def qdma(eng, out_, in_):
                d = eng.dma_start(out=out_, in_=in_)
                prev = qlast.get(eng.engine)
                if prev is not None:
                    add_dep_helper(d.ins, prev, sync=False, reason="queue order")
                qlast[eng.engine] = d.ins
                return d

            def bulk_dma(eng, out_, in_):
                return qdma(eng, out_, in_)

            refv = refhl[:].rearrange("w r c -> w (r c)")
            curv = curhl[:].rearrange("w r c -> w (r c)")

            band_dmas = [[] for _ in range(NCH)]

            def band_dma(ci, g, eng):
                _, n = CHUNKS[ci]
                r2 = R2_OF[ci]
                Mc = n * MROW
                bt = bandt[ci]
                run = SPAN * NBAND * r2
                dst = bt[:, 0 : n * SPAN * NBAND].rearrange(
                    "p (a j) -> p a j", j=SPAN * NBAND
                )
                dst.ap[0] = [Mc, PB]
                dst.ap[1] = [MROW * r2, n // r2]
                dst.ap[2] = [1, run]
                dst.offset = PB * g * Mc + NBAND * PB * g * r2
                gsz = PB * n * SPAN * NBAND
                src = qb[qoffs[ci] + g * gsz : qoffs[ci] + (g + 1) * gsz]
                src = src.rearrange("(p a j) -> p a j", p=PB, j=run)
                d = qdma(eng, dst, src)
                band_dmas[ci].append(d.ins)

            # input stream; HWDGE queues are FIFO per engine so order
            # matters: chunk-0 bands + near ref rows first.
            def chunk_bands(ci):
                for g in range(NG // 2):
                    band_dma(ci, g, nc.sync)
                for g in range(NG // 2, NG):
                    band_dma(ci, g, nc.scalar)

            # DMA emission is interleaved with compute (one-chunk
            # lookahead): Tile's 8-lane HWDGE semaphore accounting makes a
            # consumer wait on the cumulative lane count at its scheduled
            # position, so band DMAs emitted far ahead of their chunk's
            # matmuls would stall the PE on unrelated later transfers.
            qdma(nc.sync, refsb[:, 0 : 12 * C], refv[:, 0 : 12 * C])
            qdma(nc.scalar, cursb[:, 0 : 8 * C], curv[:, 0 : 8 * C])
            chunk_bands(0)
            bulk_dma(
                nc.sync, refsb[:, 12 * C : 36 * C], refv[:, 12 * C : 36 * C]
            )
            chunk_bands(1)

            def pre_chunk_inputs(ci):
                # strict need-order: ref rows 36+ are first touched by chunk
                # 4 (row 32), cur rows only by the (lagging) evictions.
                if ci == 1:
                    chunk_bands(2)
                    bulk_dma(
                        nc.scalar, cursb[:, 8 * C : 24 * C], curv[:, 8 * C : 24 * C]
                    )
                elif ci == 2:
                    chunk_bands(3)
                elif ci == 3:
                    bulk_dma(nc.sync, refsb[:, 36 * C :], refv[:, 36 * C :])
                    bulk_dma(
                        nc.scalar, cursb[:, 24 * C : 40 * C], curv[:, 24 * C : 40 * C]
                    )
                    chunk_bands(4)
                elif ci == 4:
                    bulk_dma(nc.scalar, cursb[:, 40 * C :], curv[:, 40 * C :])
                    chunk_bands(5)
                    # rows 0-31 are evicted by now; stream their output
                    # blocks behind the last input chunk
                    out_dma(nc.sync, 0)
                    out_dma(nc.scalar, 1)

            def out_dma(eng, m):
                lo, hi = m * 16, (m + 1) * 16
                qdma(
                    eng,
                    out[:, lo:hi, :].rearrange("w r c -> w (r c)"),
                    outst[:, lo * C:hi * C],
                )

            # out block ready after the chunk that completes its rows
            out_after_chunk = {2: (nc.sync, 0), 3: (nc.scalar, 1),
                               4: (nc.sync, 2), 5: (nc.scalar, 3)}

            last_mm = [None] * NCH
            for ci in range(NCH):
                pre_chunk_inputs(ci)
                h0, n = CHUNKS[ci]
                bt = bandt[ci]
                for rp in range(n // 2):
                    ps = psum_pool.tile([W, 2 * C], F32, tag="ps")
                    for sub in range(2):
                        r = 2 * rp + sub
                        h = h0 + r
                        r2 = R2_OF[ci]
                        rg, rr = divmod(r, r2)
                        for b in range(NBAND):
                            lhsT = bt[
                                :, rg * MROW * r2 : (rg + 1) * MROW * r2
                            ].rearrange(
                                "p (q five rr) -> p q five rr",
                                five=NBAND,
                                rr=r2,
                            )[:, 2:130, b, rr]
                            rhs = refsb[:, (h + b) * C : (h + b + 1) * C]
                            mm = nc.tensor.matmul(
                                ps[:, sub * C : (sub + 1) * C],
                                lhsT,
                                rhs,
                                start=(b == 0),
                                stop=(b == NBAND - 1),
                            )
                            last_mm[ci] = mm.ins
                    # evict both rows, adding the host-prescaled current term
                    he = h0 + 2 * rp
                    nc.vector.tensor_add(
                        outst[:, he * C : (he + 2) * C],
                        ps[:],
                        cursb[:, he * C : (he + 2) * C],
                    )

            # remaining outputs drain at the end
            for m in (2, 3):
                out_dma(nc.sync if m % 2 == 0 else nc.scalar, m)

    return nc


_NC = None
LAST_RESULT = None


def _get_nc():
    global _NC
    if _NC is None:
        _NC = _build_bass()
    return _NC


# ---------------------------------------------------------------------------
# Host-side shard prep
# ---------------------------------------------------------------------------


def _prep_core(attn_b, rv_b, cv_b, g0):
    """Build one core's in_map. attn_b/rv_b/cv_b: [H, W, ...] for one batch;
    g0: first output row of the shard."""
    # ref with 2-row halo, transposed to [w, r, c]
    refpad = np.zeros((HALO_R, W, C), np.float32)
    lo_g, hi_g = g0 - 2, g0 + HS + 2
    s0, s1 = max(lo_g, 0), min(hi_g, H)
    refpad[s0 - lo_g : s1 - lo_g] = rv_b[s0:s1]
    refhl = refpad.transpose(1, 0, 2).astype(bfloat16)  # [w, r, c]

    # current term pre-scaled by its attention weight (fp32 on host)
    cur = cv_b[g0 : g0 + HS] * attn_b[g0 : g0 + HS, :, 25:26]
    curhl = cur.transpose(1, 0, 2).astype(bfloat16)  # [w, h, c]

    # V2[r, w', j, b] = attn[r, w'+j-2, 5b+4-j] (0 outside the image)
    A = attn_b[g0 : g0 + HS]  # [HS, W, 26]
    wv = np.arange(W)
    jv = np.arange(5)
    b5 = np.arange(5)
    Wi = wv[:, None] + jv[None, :] - 2            # [128, 5]
    valid = (Wi >= 0) & (Wi < W)
    Wc = np.clip(Wi, 0, W - 1)
    Kidx = 5 * b5[None, :] + 4 - jv[:, None]      # [5(j), 5(b)]
    V = A[:, Wc, :]                               # [HS, 128, 5, 26]
    V2 = np.take_along_axis(V, Kidx[None, None, :, :], axis=3)  # [HS,128,5,5]
    V2 = (V2 * valid[None, :, :, None]).astype(np.float32)
    V2 = V2.reshape(HS, NG, PB, 5, 5)             # [r, g, p^, j, b]

    # scatter into span windows: window index i = p^ + j
    q = np.zeros((HS, NG, PB, SPAN, NBAND), np.float32)
    pv = np.arange(PB)
    for j in range(5):
        q[:, :, pv, pv + j, :] = V2[:, :, pv, j, :]

    parts = []
    for ci, (h0, n) in enumerate(CHUNKS):
        r2 = R2_OF[ci]
        blk = q[h0 : h0 + n]                      # [n, g, p^, i, b]
        blk = blk.reshape(n // r2, r2, NG, PB, SPAN, NBAND)
        blk = blk.transpose(2, 3, 0, 4, 5, 1)     # [g, p^, rg, i, b, rr]
        parts.append(blk.reshape(-1))
    qbv = np.concatenate(parts).astype(bfloat16)
    return {"refhl": refhl, "curhl": curhl, "qb": qbv}


def kernel(attn, ref_value, current_ref_value):
    attn = np.asarray(attn, dtype=np.float32)
    rv = np.asarray(ref_value, dtype=np.float32)
    cv = np.asarray(current_ref_value, dtype=np.float32)

    nc = _get_nc()
    in_maps = []
    for core in range(NCORES):
        bb, half = divmod(core, 2)
        in_maps.append(_prep_core(attn[bb], rv[bb], cv[bb], half * HS))

    res = bass_utils.run_bass_kernel_spmd(nc, in_maps, core_ids=list(range(NCORES)))
    global LAST_RESULT
    LAST_RESULT = res

    out = np.empty((B, H, W, C), np.float32)
    for core in range(NCORES):
        bb, half = divmod(core, 2)
        dev = res.results[core]["out"]  # [w, hs, c] bf16
        out[bb, half * HS : (half + 1) * HS] = dev.transpose(1, 0, 2).astype(
            np.float32
        )
    return out

